# revision 1
# baseline (speedup 1.0000x reference)
"""Inverse separable wavelet synthesis (stride-2 transposed conv, 9 taps,
36 -> 12 -> 4 channels, 256x256 -> 512x512) on 8 trn2 NeuronCores.

Formulation: both passes are dense matmuls against host-precomputed banded
operator matrices A_beta [256 in, 512 out] (one per wavelet band), with
symmetric padding + border-mask sign folded into the operators.  All filter
coefficients are dyadic rationals with <= 8 mantissa bits -> EXACT in bf16,
so everything on-chip runs in bf16 (input and intermediates round to bf16;
PSUM accumulation stays exact fp32).

Host side: input is permuted to [b, h, by, g2, bx, w] (band 'by' outermost
of the channel split c = 9*g2 + 3*by + bx) and cast to bf16.  This makes
every matmul moving-operand access pattern contiguous in 256-byte runs
(full 16B-SBUF-cacheline hits) and halves input DMA bytes.

Input DMA descriptors are split to 4608 B (max_dma_last_dim) so each load
instruction carries 272 descriptors -> the HWDGE spreads them over all 16
SDMA engines (68 x 18KB descriptors land on only 4 engines = the previous
bottleneck).

Per-core pipeline (pure batch parallelism, 2 images per core), fully
streamed per 128-row h2-block:
  load x window [68 h, (by g2 bx w)]  (one DMA, 272 descriptors)
  stage Y : lhsT = A_by window [68, 128 h2], moving = x win [68, (g2, w)]
            -> psY [h2, (g2, w-half)] (3-band accumulation), cast -> u bf16
  PE transpose u [h2, (q, w)] -> up [w, (q, h2)]      (q = 3*g2 + bx)
  stage X : lhsT = A_bx block [128 w, 128 w2], moving = up [w, (g2, h2)]
            -> psX [w2, (g2, h2)], cast -> v bf16
  PE transpose v [w2, (c, h2)] -> osb [h2, (w2, c)] fp32 -> DRAM rows
"""

import numpy as np
import ml_dtypes
from contextlib import ExitStack

import concourse.bass as bass
import concourse.bacc as bacc
import concourse.mybir as mybir
import concourse.tile as tile
from concourse.bass_utils import run_bass_kernel_spmd

B, H, W, C = 16, 256, 256, 36
NCORES = 8
BPC = B // NCORES  # batches per core
W2 = 2 * W
H2 = 2 * H
F32 = mybir.dt.float32
BF16 = mybir.dt.bfloat16

SMOOTH = [0.0, 0.0, 1.0 / 16.0, 0.5, 14.0 / 16.0, 0.5, 1.0 / 16.0, 0.0, 0.0]
EVEN = [-1.0 / 128.0, -1.0 / 16.0, -10.0 / 64.0, -7.0 / 16.0, 85.0 / 64.0,
        -7.0 / 16.0, -10.0 / 64.0, -1.0 / 16.0, -1.0 / 128.0]
ODD = [1.0 / 256.0, 1.0 / 32.0, 15.0 / 128.0, 17.0 / 32.0, 0.0,
       -17.0 / 32.0, -15.0 / 128.0, -1.0 / 32.0, -1.0 / 256.0]

# Stage X: which 128-row k-tiles of up feed each 128-col w2 output block
# (out block n covers in rows [64n-2, 64n+65]).
KTS = {0: (0,), 1: (0, 1), 2: (0, 1), 3: (1,)}
# Stage Y input windows (one 70-row window per 128-row h2 output block).
# Uniform stride 62 lets ONE DMA instruction load all 4 windows (280
# descriptors -> spread over all 16 SDMA engines; 68-descriptor loads
# land on only 4 engines and bottleneck the kernel).
W0 = [0, 62, 124, 186]
KW = 70


def _build_operator_full():
    """[3 bands, 256 in-rows, 512 out-cols] float64 folded operator."""
    inv = np.array([SMOOTH, EVEN, ODD], dtype=np.float64)
    S = 256
    Sp = S + 6
    j = np.arange(Sp)[:, None]
    m = np.arange(2 * S)[None, :]
    t = m + 10 - 2 * j
    valid = (t >= 0) & (t <= 8)
    P = np.zeros((3, Sp, 2 * S))
    for b in range(3):
        P[b][valid] = inv[b][t[valid]]
    # border mask: odd band negated on the 3-wide padded border
    P[2, [0, 1, 2, Sp - 3, Sp - 2, Sp - 1], :] *= -1.0
    # fold symmetric padding: pad[0..2]=x[2],x[1],x[0]; pad[-3:]=x[-1],x[-2],x[-3]
    A = P[:, 3:3 + S].copy()
    A[:, 2] += P[:, 0]
    A[:, 1] += P[:, 1]
    A[:, 0] += P[:, 2]
    A[:, S - 1] += P[:, Sp - 3]
    A[:, S - 2] += P[:, Sp - 2]
    A[:, S - 3] += P[:, Sp - 1]
    return A


def _build_operator_array():
    """Stage-X operator: [3 bands, 2 ktiles, 128 in-rows, 512 out-cols] bf16."""
    A = _build_operator_full()
    return np.ascontiguousarray(
        A.reshape(3, 2, 128, 512).astype(ml_dtypes.bfloat16))


def _build_operator_windows():
    """Stage-Y operator: [3 bands, 4 blocks, 68 in-rows, 128 out-cols] bf16."""
    A = _build_operator_full()
    out = np.zeros((3, 4, KW, 128), np.float64)
    for blk in range(4):
        out[:, blk] = A[:, W0[blk]:W0[blk] + KW, blk * 128:(blk + 1) * 128]
    return np.ascontiguousarray(out.astype(ml_dtypes.bfloat16))


def _build_program(repeat=1):
    nc = bacc.Bacc("TRN2", target_bir_lowering=False)
    # x: [b, p, win, row(+pad)] bf16 — host-materialized overlapping stage-Y
    # windows (row = (by, g2, bx, w) channel-permuted).  Row-interleaved
    # window layout keeps the DMA descriptor stream sequential in DRAM;
    # the 128 B pad stops descriptor coalescing so one load = 280 x 18 KB
    # descriptors -> all 16 SDMA engines at full rate.
    ROW = 3 * 12 * W
    PADW = 64
    x = nc.declare_dram_parameter("x", [BPC, 2, KW, 2, ROW + PADW], BF16,
                                  isOutput=False)
    a_w = nc.declare_dram_parameter("a_w", [3, 4, KW, 128], BF16,
                                    isOutput=False)
    a_op = nc.declare_dram_parameter("a_op", [3, 2, 128, W2], BF16,
                                     isOutput=False)
    ident = nc.declare_dram_parameter("ident", [128, 128], BF16,
                                      isOutput=False)
    out = nc.declare_dram_parameter("out", [BPC, H2, W2, 4], F32,
                                    isOutput=True)

    with tile.TileContext(nc) as tc, ExitStack() as ctx:
        const = ctx.enter_context(tc.tile_pool(name="const", bufs=1))
        xpoolA = ctx.enter_context(tc.tile_pool(name="xpA", bufs=2))
        xpoolB = ctx.enter_context(tc.tile_pool(name="xpB", bufs=1))
        upool = ctx.enter_context(tc.tile_pool(name="up", bufs=2))
        wpool = ctx.enter_context(tc.tile_pool(name="wp", bufs=2))
        vpool = ctx.enter_context(tc.tile_pool(name="vp", bufs=2))
        opool = ctx.enter_context(tc.tile_pool(name="op", bufs=3))
        psY = ctx.enter_context(tc.tile_pool(name="psY", bufs=2, space="PSUM"))
        psT = ctx.enter_context(tc.tile_pool(name="psT", bufs=2, space="PSUM"))
        psX = ctx.enter_context(tc.tile_pool(name="psX", bufs=2, space="PSUM"))
        psO = ctx.enter_context(tc.tile_pool(name="psO", bufs=2, space="PSUM"))

        aw_sb = {}
        for beta in range(3):
            for blk in range(4):
                t = const.tile([KW, 128], BF16, name=f"aw_{beta}_{blk}",
                               tag=f"aw_{beta}_{blk}")
                nc.sync.dma_start(t[:], a_w[beta, blk])
                aw_sb[beta, blk] = t
        a_sb = {}
        for beta in range(3):
            for kt in range(2):
                t = const.tile([128, W2], BF16, name=f"a_{beta}_{kt}",
                               tag=f"a_{beta}_{kt}")
                nc.sync.dma_start(t[:], a_op[beta, kt])
                a_sb[beta, kt] = t
        ident_sb = const.tile([128, 128], BF16, name="ident_sb", tag="ident")
        nc.sync.dma_start(ident_sb[:], ident[:])

        for rep in range(repeat):
          for b in range(BPC):
            rb = rep * BPC + b
            # ---- two DMAs per image (window pairs), 140 descriptors each
            xts = {}
            for pr, pool in ((0, xpoolA), (1, xpoolB)):
                xt = pool.tile([KW, 2 * ROW], BF16, name=f"x_{rb}_{pr}",
                               tag=f"x{pr}")
                src = x[b, pr, :, :, 0:ROW]
                dst = xt.rearrange("h (win r) -> h win r", win=2)
                eng = nc.sync if pr == 0 else nc.scalar
                eng.dma_start(dst, src)
                xts[pr] = xt
            for blk in range(4):
                xv = xts[blk // 2].rearrange(
                    "h (win by g2 bx w) -> h win by g2 bx w",
                    win=2, by=3, g2=4, bx=3)[:, blk % 2]

                # ---- stage Y: u[blk] [h2 128, (g2, bx, w)] bf16
                u = upool.tile([128, 12 * W], BF16, name=f"u_{rb}_{blk}",
                               tag="u")
                uv = u.rearrange("p (g2 bx w) -> p g2 bx w", g2=4, bx=3)
                for bx in range(3):
                    for wc in range(2):
                        ps = psY.tile([128, 512], F32,
                                      name=f"psY_{rb}_{blk}_{bx}_{wc}",
                                      tag="psY")
                        psv = ps.rearrange("p (g w) -> p g w", g=4)
                        for i, by in enumerate(range(3)):
                            rhs = xv[:, by, :, bx, wc * 128:(wc + 1) * 128]
                            nc.tensor.matmul(psv, aw_sb[by, blk][:], rhs,
                                             start=(i == 0), stop=(i == 2))
                        dst = uv[:, :, bx, wc * 128:(wc + 1) * 128]
                        if (bx + wc) % 2 == 0:
                            nc.vector.tensor_copy(out=dst, in_=psv)
                        else:
                            nc.scalar.copy(out=dst, in_=psv)

                # ---- mid transpose: u [h2, (q, w)] -> up[wt] [w, (q, h2)]
                up = {}
                for wt in range(2):
                    up[wt] = wpool.tile([128, 12 * 128], BF16,
                                        name=f"up_{rb}_{blk}_{wt}", tag="upt")
                uvb = u.rearrange("p (q w) -> p q w", q=12)
                for wt in range(2):
                    for q4 in range(3):
                        pt = psT.tile([128, 512], BF16,
                                      name=f"psT_{rb}_{blk}_{wt}_{q4}",
                                      tag="psT")
                        for i in range(4):
                            q = q4 * 4 + i
                            nc.tensor.transpose(
                                pt[:, i * 128:(i + 1) * 128],
                                uvb[:, q, wt * 128:(wt + 1) * 128],
                                ident_sb[:])
                        dst = up[wt].rearrange("p (q h) -> p q h", q=12)[
                            :, q4 * 4:(q4 + 1) * 4, :]
                        src2 = pt.rearrange("p (q h) -> p q h", q=4)
                        if (wt + q4) % 2 == 0:
                            nc.vector.tensor_copy(out=dst, in_=src2)
                        else:
                            nc.scalar.copy(out=dst, in_=src2)

                # ---- stage X for this h2 block -> v[w2b] [w2, (c, h2slice)]
                v = {}
                for w2b in range(4):
                    ps = psX.tile([128, 512], F32,
                                  name=f"psX_{rb}_{blk}_{w2b}", tag="psX")
                    psv = ps.rearrange("p (g h) -> p g h", g=4)
                    mms = [(bx, kt) for bx in range(3) for kt in KTS[w2b]]
                    for i, (bx, kt) in enumerate(mms):
                        lhsT = a_sb[bx, kt][:, w2b * 128:(w2b + 1) * 128]
                        # up free order q = 3*g2 + bx -> fix bx, stride g2
                        rhs = up[kt].rearrange(
                            "p (g2 e h) -> p e g2 h", g2=4, e=3)[:, bx, :, :]
                        nc.tensor.matmul(psv, lhsT, rhs,
                                         start=(i == 0),
                                         stop=(i == len(mms) - 1))
                    vt = vpool.tile([128, 4 * 128], BF16,
                                    name=f"v_{rb}_{blk}_{w2b}",
                                    tag=f"v_{w2b}")
                    dst = vt.rearrange("p (c h) -> p c h", c=4)
                    if w2b % 2 == 0:
                        nc.scalar.copy(out=dst, in_=psv)
                    else:
                        nc.vector.tensor_copy(out=dst, in_=psv)
                    v[w2b] = vt

                # ---- output transpose: v [w2, (c, h2)] -> osb [h2, (w2, c)]
                osb = opool.tile([128, W2 * 4], F32, name=f"osb_{rb}_{blk}",
                                 tag="osb")
                osbv = osb.rearrange("p (w c) -> p c w", c=4)
                for w2b in range(4):
                    pt = psO.tile([128, 512], BF16,
                                  name=f"psO_{rb}_{blk}_{w2b}", tag="psO")
                    vv = v[w2b].rearrange("p (c h) -> p c h", c=4)
                    for c in range(4):
                        nc.tensor.transpose(
                            pt[:, c * 128:(c + 1) * 128],
                            vv[:, c, :],
                            ident_sb[:])
                    dst = osbv[:, :, w2b * 128:(w2b + 1) * 128]
                    src2 = pt.rearrange("p (c w) -> p c w", c=4)
                    if w2b % 2 == 0:
                        nc.vector.tensor_copy(out=dst, in_=src2)
                    else:
                        nc.scalar.copy(out=dst, in_=src2)
                dstd = out[b, blk * 128:(blk + 1) * 128, :, :].rearrange(
                    "h w c -> h (w c)")
                nc.sync.dma_start(dstd, osb[:])
    nc.compile()
    return nc


_PROGRAMS = {}


def _get_program(repeat=1):
    if repeat not in _PROGRAMS:
        _PROGRAMS[repeat] = _build_program(repeat)
    return _PROGRAMS[repeat]


def _host_inputs(inputs):
    a4 = _build_operator_array()
    aw = _build_operator_windows()
    identity = np.ascontiguousarray(np.eye(128, dtype=ml_dtypes.bfloat16))
    # [B,H,W,C] c = 9*g2 + 3*by + bx -> rows [B, H, (by, g2, bx, W)] bf16
    xp = inputs.reshape(B, H, W, 4, 3, 3).transpose(0, 1, 4, 3, 5, 2)
    xp = xp.astype(ml_dtypes.bfloat16).reshape(B, H, 3 * 12 * W)
    ROW = 3 * 12 * W
    PADW = 64
    # materialize the 4 overlapping stage-Y windows, pair-grouped and
    # row-interleaved within each pair, plus pad to stop desc coalescing
    idx = np.arange(KW)[:, None] + np.asarray(W0)[None, :]  # [KW, 4]
    xw = np.zeros((B, 2, KW, 2, ROW + PADW), dtype=ml_dtypes.bfloat16)
    gathered = xp[:, idx, :]  # [B, KW, 4, ROW]
    xw[:, 0, :, :, :ROW] = gathered[:, :, 0:2]
    xw[:, 1, :, :, :ROW] = gathered[:, :, 2:4]
    shards = xw.reshape(NCORES, BPC, 2, KW, 2, ROW + PADW)
    return [{"x": np.ascontiguousarray(shards[c]), "a_op": a4, "a_w": aw,
             "ident": identity} for c in range(NCORES)]


def _run(inputs, trace=False, tmpdir=None, repeat=1):
    """Returns (full output [16,512,512,4], BassKernelResults)."""
    inputs = np.ascontiguousarray(np.asarray(inputs, dtype=np.float32))
    assert inputs.shape == (B, H, W, C), inputs.shape
    nc = _get_program(repeat)
    in_maps = _host_inputs(inputs)
    res = run_bass_kernel_spmd(nc, in_maps, core_ids=list(range(NCORES)),
                               trace=trace, tmpdir=tmpdir)
    outs = [np.asarray(res.results[c]["out"]) for c in range(NCORES)]
    full = np.concatenate(outs, axis=0).astype(np.float32)
    return full, res


def kernel(inputs):
    full, _ = _run(inputs)
    return full



# revision 6
# speedup vs baseline: 1.5520x; 1.5520x over previous
"""Inverse separable wavelet synthesis (stride-2 transposed conv, 9 taps,
36 -> 12 -> 4 channels, 256x256 -> 512x512) on 8 trn2 NeuronCores.

X-FIRST dense-operator formulation (v2).  Both passes are matmuls against
the same host-precomputed banded operator A [256 in, 512 out] (one per
wavelet band, symmetric padding + border-mask sign folded in).  All
coefficients are dyadic rationals exact in bf16.

Pass 1 (X, along width) runs FIRST (reference order), contraction over
(bx band, w window) jointly: the host stacks the 3 bands' 70-row windows
into two 105-row chunks, so each 128-col w2 output block needs only TWO
matmul pumps (vs 4.5 for tile-aligned k + per-band pumps).

  t[w2, (by,g2,h)] = sum_{bx,w} A[bx,w,w2] x[h,w,c]     (48 mm/img)

Mid transpose on the PE (identity trick), tile-aligned 128x128:

  u2[by,kt][h, (g2,w2)] = t^T                           (96 tr/img)

Pass 2 (Y, along height) contracts h k-tiles (KTS banding, 1-2 tiles per
128-row h2 block) and lands DIRECTLY in output row layout -> no output
transpose at all:

  out[h2, (g2,w2)] = sum_{by,kt} A[by,kt,h2] u2         (72 mm/img)

Output is stored bf16 [b, h2, g2, w2] (halves store bytes); the host
transposes (g2,w2)->(w2,g2) and upcasts to fp32.

DMA: input is host-materialized as band-stacked chunk windows, split by
h-half so compute starts after ~1.3 MB instead of ~5 MB; all x loads go
in exact consumption order on the sync-engine queue (the baseline lost
~40 us to a later-needed load winning DMA-engine arbitration over the
first-needed one).  Constants go on the scalar-engine queue in parallel;
output stores on the scalar queue.  ~25 junk transposes warm the PE's
HAM clock (1.2->2.4 GHz) before the first real matmul.
"""

import numpy as np
import ml_dtypes
from contextlib import ExitStack

import concourse.bass as bass
import concourse.bacc as bacc
import concourse.mybir as mybir
import concourse.tile as tile
from concourse.bass_utils import run_bass_kernel_spmd

B, H, W, C = 16, 256, 256, 36
NCORES = 8
BPC = B // NCORES  # batches per core
W2 = 2 * W
H2 = 2 * H
F32 = mybir.dt.float32
BF16 = mybir.dt.bfloat16

SMOOTH = [0.0, 0.0, 1.0 / 16.0, 0.5, 14.0 / 16.0, 0.5, 1.0 / 16.0, 0.0, 0.0]
EVEN = [-1.0 / 128.0, -1.0 / 16.0, -10.0 / 64.0, -7.0 / 16.0, 85.0 / 64.0,
        -7.0 / 16.0, -10.0 / 64.0, -1.0 / 16.0, -1.0 / 128.0]
ODD = [1.0 / 256.0, 1.0 / 32.0, 15.0 / 128.0, 17.0 / 32.0, 0.0,
       -17.0 / 32.0, -15.0 / 128.0, -1.0 / 32.0, -1.0 / 256.0]

# Which 128-row k-tiles of u2 feed each 128-col h2 output block
# (out block n depends on input rows [64n-2, 64n+65]).
KTS = {0: (0,), 1: (0, 1), 2: (0, 1), 3: (1,)}
# Stage-X 70-row input windows per 128-col w2 output block.
W0 = [0, 62, 124, 186]
KW = 70
HKW = KW // 2  # 35

# x row layout: free = (by 3, g2 4, h 128) per h-half = 1536 elems; pad to
# 1600 and load only 1568 so descriptors (3136 B) never coalesce.
FREE = 1536
FREEP = 1600
FREEL = 1568


def _build_operator_full():
    """[3 bands, 256 in-rows, 512 out-cols] float64 folded operator."""
    inv = np.array([SMOOTH, EVEN, ODD], dtype=np.float64)
    S = 256
    Sp = S + 6
    j = np.arange(Sp)[:, None]
    m = np.arange(2 * S)[None, :]
    t = m + 10 - 2 * j
    valid = (t >= 0) & (t <= 8)
    P = np.zeros((3, Sp, 2 * S))
    for b in range(3):
        P[b][valid] = inv[b][t[valid]]
    # border mask: odd band negated on the 3-wide padded border
    P[2, [0, 1, 2, Sp - 3, Sp - 2, Sp - 1], :] *= -1.0
    # fold symmetric padding: pad[0..2]=x[2],x[1],x[0]; pad[-3:]=x[-1],x[-2],x[-3]
    A = P[:, 3:3 + S].copy()
    A[:, 2] += P[:, 0]
    A[:, 1] += P[:, 1]
    A[:, 0] += P[:, 2]
    A[:, S - 1] += P[:, Sp - 3]
    A[:, S - 2] += P[:, Sp - 2]
    A[:, S - 3] += P[:, Sp - 1]
    return A


def _build_ay():
    """Stage-Y operator: [3 bands, 2 ktiles, 128 in-rows, 512 out-cols] bf16."""
    A = _build_operator_full()
    return np.ascontiguousarray(
        A.reshape(3, 2, 128, 512).astype(ml_dtypes.bfloat16))


def _build_ax():
    """Stage-X chunked operator: [4 w2blk, 2 chunks, 105, 128] bf16.

    chunk0 rows = (bx=0, win 70) + (bx=1, win[:35]);
    chunk1 rows = (bx=1, win[35:]) + (bx=2, win 70).
    """
    A = _build_operator_full()
    out = np.zeros((4, 2, KW + HKW, 128), np.float64)
    for n in range(4):
        w = slice(W0[n], W0[n] + KW)
        cols = slice(n * 128, (n + 1) * 128)
        out[n, 0, :KW] = A[0, w, cols]
        out[n, 0, KW:] = A[1, W0[n]:W0[n] + HKW, cols]
        out[n, 1, :HKW] = A[1, W0[n] + HKW:W0[n] + KW, cols]
        out[n, 1, HKW:] = A[2, w, cols]
    return np.ascontiguousarray(out.astype(ml_dtypes.bfloat16))


def _build_program(repeat=1):
    nc = bacc.Bacc("TRN2", target_bir_lowering=False)
    # x: [b, hhalf, w2blk, chunk, 105 rows, FREEP] bf16, row = (by,g2,h128)
    x = nc.declare_dram_parameter("x", [BPC, 2, 4, 2, KW + HKW, FREEP], BF16,
                                  isOutput=False)
    a_y = nc.declare_dram_parameter("a_y", [3, 2, 128, 512], BF16,
                                    isOutput=False)
    a_x = nc.declare_dram_parameter("a_x", [4, 2, KW + HKW, 128], BF16,
                                    isOutput=False)
    ident = nc.declare_dram_parameter("ident", [128, 128], BF16,
                                      isOutput=False)
    # out rows are (g2, w2) -- host swaps to (w2, g2) and upcasts
    out = nc.declare_dram_parameter("out", [BPC, H2, 4, W2], BF16,
                                    isOutput=True)

    with tile.TileContext(nc) as tc, ExitStack() as ctx:
        const = ctx.enter_context(tc.tile_pool(name="const", bufs=1))
        xpool = ctx.enter_context(tc.tile_pool(name="xp", bufs=6))
        tpool = ctx.enter_context(tc.tile_pool(name="tp", bufs=10))
        upool = ctx.enter_context(tc.tile_pool(name="up", bufs=8))
        opool = ctx.enter_context(tc.tile_pool(name="op", bufs=3))
        psX = ctx.enter_context(tc.tile_pool(name="psX", bufs=2, space="PSUM"))
        psT = ctx.enter_context(tc.tile_pool(name="psT", bufs=2, space="PSUM"))
        psY = ctx.enter_context(tc.tile_pool(name="psY", bufs=2, space="PSUM"))

        # ---- constants: scalar-engine DMA queue (parallel to x stream)
        ax_sb = {}
        for n in range(4):
            for c in range(2):
                t = const.tile([KW + HKW, 128], BF16, name=f"ax_{n}_{c}",
                               tag=f"ax_{n}_{c}")
                nc.scalar.dma_start(t[:], a_x[n, c])
                ax_sb[n, c] = t
        ident_sb = const.tile([128, 128], BF16, name="ident_sb", tag="ident")
        nc.scalar.dma_start(ident_sb[:], ident[:])
        ay_sb = {}
        for by in range(3):
            for kt in range(2):
                t = const.tile([128, 512], BF16, name=f"ay_{by}_{kt}",
                               tag=f"ay_{by}_{kt}")
                nc.scalar.dma_start(t[:], a_y[by, kt])
                ay_sb[by, kt] = t

        # ---- input loads: sync-engine queue, exact consumption order
        xts = {}
        for rep in range(repeat):
            for b in range(BPC):
                for hh in range(2):
                    for n in range(4):
                        xt = xpool.tile([KW + HKW, 2 * FREEL], BF16,
                                        name=f"x_{rep}_{b}_{hh}_{n}",
                                        tag="x")
                        dst = xt.rearrange("p (c f) -> p c f", c=2)
                        srcv = x[b, hh, n].rearrange(
                            "c p f -> p c f")[:, :, 0:FREEL]
                        nc.sync.dma_start(dst, srcv)
                        xts[rep, b, hh, n] = xt

        # ---- PE warmup: junk transposes to lift HAM clock before data lands
        for i in range(25):
            pw = psT.tile([128, 512], BF16, name=f"warm_{i}", tag="psT")
            nc.tensor.transpose(pw[:, 0:128], ident_sb[:], ident_sb[:])

        ncopy = 0

        def copy(dst, src):
            nonlocal ncopy
            ncopy += 1
            if ncopy % 2 == 0:
                nc.vector.tensor_copy(out=dst, in_=src)
            else:
                nc.scalar.copy(out=dst, in_=src)

        for rep in range(repeat):
          for b in range(BPC):
            rb = rep * BPC + b
            tsb = {}   # (hh, w2t) -> [128 w2, (q 12, h 128)]
            u2 = {}    # (by, kt)  -> [128 h, (g2 4, w2 512)]

            def stage_x(hh):
                for n in range(4):
                    xv = xts[rep, b, hh, n].rearrange(
                        "p (c f) -> p c f", c=2)
                    tt = tpool.tile([128, FREE], BF16,
                                    name=f"t_{rb}_{hh}_{n}",
                                    tag="t")
                    for t3 in range(3):
                        ps = psX.tile([128, 512], F32,
                                      name=f"psX_{rb}_{hh}_{n}_{t3}",
                                      tag="psX")
                        for c in range(2):
                            rhs = xv[:, c, t3 * 512:(t3 + 1) * 512]
                            nc.tensor.matmul(ps[:], ax_sb[n, c][:], rhs,
                                             start=(c == 0), stop=(c == 1))
                        copy(tt[:, t3 * 512:(t3 + 1) * 512], ps[:])
                    tsb[hh, n] = tt

            def stage_t(kt):
                for by in range(3):
                    ut = upool.tile([128, 2048], BF16,
                                    name=f"u_{rb}_{by}_{kt}",
                                    tag="u")
                    for g2 in range(4):
                        q = by * 4 + g2
                        pt = psT.tile([128, 512], BF16,
                                      name=f"psT_{rb}_{kt}_{by}_{g2}",
                                      tag="psT")
                        for w2t in range(4):
                            in_ = tsb[kt, w2t][:, q * 128:(q + 1) * 128]
                            nc.tensor.transpose(
                                pt[:, w2t * 128:(w2t + 1) * 128],
                                in_, ident_sb[:])
                        # u2 free layout (g2, w2): contiguous dst -> 2x DVE
                        copy(ut[:, g2 * 512:(g2 + 1) * 512], pt[:])
                    u2[by, kt] = ut

            def stage_y(n):
                osb = opool.tile([128, 4 * 512], BF16,
                                 name=f"osb_{rb}_{n}", tag="osb")
                for g2 in range(4):
                    ps = psY.tile([128, 512], F32,
                                  name=f"psY_{rb}_{n}_{g2}", tag="psY")
                    mms = [(by, kt) for by in range(3) for kt in KTS[n]]
                    for i, (by, kt) in enumerate(mms):
                        lhsT = ay_sb[by, kt][:, n * 128:(n + 1) * 128]
                        rhs = u2[by, kt][:, g2 * 512:(g2 + 1) * 512]
                        nc.tensor.matmul(ps[:], lhsT, rhs,
                                         start=(i == 0),
                                         stop=(i == len(mms) - 1))
                    copy(osb[:, g2 * 512:(g2 + 1) * 512], ps[:])
                dstd = out[b, n * 128:(n + 1) * 128, :, :].rearrange(
                    "h g w -> h (g w)")
                nc.scalar.dma_start(dstd, osb[:])

            stage_x(0)
            stage_t(0)
            stage_y(0)
            stage_x(1)
            stage_t(1)
            stage_y(1)
            stage_y(2)
            stage_y(3)
    nc.compile()
    return nc


_PROGRAMS = {}


def _get_program(repeat=1):
    if repeat not in _PROGRAMS:
        _PROGRAMS[repeat] = _build_program(repeat)
    return _PROGRAMS[repeat]


def _host_inputs(inputs):
    ax = _build_ax()
    ay = _build_ay()
    identity = np.ascontiguousarray(np.eye(128, dtype=ml_dtypes.bfloat16))
    # [B,H,W,C] c = 9*g2 + 3*by + bx -> xt [b, bx, w, by, g2, h] bf16
    xt = inputs.reshape(B, H, W, 4, 3, 3).transpose(0, 5, 2, 4, 3, 1)
    xt = np.ascontiguousarray(xt).astype(ml_dtypes.bfloat16)
    # band-stacked chunk windows, h-halved:
    # xw [b, hh, w2blk, chunk, 105, (by, g2, h128)] (+pad)
    xw = np.zeros((B, 2, 4, 2, KW + HKW, FREEP), dtype=ml_dtypes.bfloat16)
    xr = xw[..., :FREE].reshape(B, 2, 4, 2, KW + HKW, 3, 4, 128)
    for n in range(4):
        w = slice(W0[n], W0[n] + KW)
        wa = slice(W0[n], W0[n] + HKW)
        wb = slice(W0[n] + HKW, W0[n] + KW)
        for hh in range(2):
            h = slice(hh * 128, (hh + 1) * 128)
            # xt slice -> [b, w, by, g2, h]
            xr[:, hh, n, 0, :KW] = xt[:, 0, w, :, :, h]
            xr[:, hh, n, 0, KW:] = xt[:, 1, wa, :, :, h]
            xr[:, hh, n, 1, :HKW] = xt[:, 1, wb, :, :, h]
            xr[:, hh, n, 1, HKW:] = xt[:, 2, w, :, :, h]
    shards = xw.reshape(NCORES, BPC, 2, 4, 2, KW + HKW, FREEP)
    return [{"x": np.ascontiguousarray(shards[c]), "a_y": ay, "a_x": ax,
             "ident": identity} for c in range(NCORES)]


def _run(inputs, trace=False, tmpdir=None, repeat=1):
    """Returns (full output [16,512,512,4], BassKernelResults)."""
    inputs = np.ascontiguousarray(np.asarray(inputs, dtype=np.float32))
    assert inputs.shape == (B, H, W, C), inputs.shape
    nc = _get_program(repeat)
    in_maps = _host_inputs(inputs)
    res = run_bass_kernel_spmd(nc, in_maps, core_ids=list(range(NCORES)),
                               trace=trace, tmpdir=tmpdir)
    outs = [np.asarray(res.results[c]["out"]) for c in range(NCORES)]
    full = np.concatenate(outs, axis=0)  # [16, 512, 4, 512] bf16
    full = full.transpose(0, 1, 3, 2).astype(np.float32)
    return np.ascontiguousarray(full), res


def kernel(inputs):
    full, _ = _run(inputs)
    return full


# revision 7
# speedup vs baseline: 1.5833x; 1.0202x over previous
"""Inverse separable wavelet synthesis (stride-2 transposed conv, 9 taps,
36 -> 12 -> 4 channels, 256x256 -> 512x512) on 8 trn2 NeuronCores.

X-FIRST dense-operator formulation (v2).  Both passes are matmuls against
the same host-precomputed banded operator A [256 in, 512 out] (one per
wavelet band, symmetric padding + border-mask sign folded in).  All
coefficients are dyadic rationals exact in bf16.

Pass 1 (X, along width) runs FIRST (reference order), contraction over
(bx band, w window) jointly: the host stacks the 3 bands' 70-row windows
into two 105-row chunks, so each 128-col w2 output block needs only TWO
matmul pumps (vs 4.5 for tile-aligned k + per-band pumps).

  t[w2, (by,g2,h)] = sum_{bx,w} A[bx,w,w2] x[h,w,c]     (48 mm/img)

Mid transpose on the PE (identity trick), tile-aligned 128x128:

  u2[by,kt][h, (g2,w2)] = t^T                           (96 tr/img)

Pass 2 (Y, along height) contracts h k-tiles (KTS banding, 1-2 tiles per
128-row h2 block) and lands DIRECTLY in output row layout -> no output
transpose at all:

  out[h2, (g2,w2)] = sum_{by,kt} A[by,kt,h2] u2         (72 mm/img)

Output is stored bf16 [b, h2, g2, w2] (halves store bytes); the host
transposes (g2,w2)->(w2,g2) and upcasts to fp32.

DMA: input is host-materialized as band-stacked chunk windows, split by
h-half so compute starts after ~1.3 MB instead of ~5 MB; all x loads go
in exact consumption order on the sync-engine queue (the baseline lost
~40 us to a later-needed load winning DMA-engine arbitration over the
first-needed one).  Constants go on the scalar-engine queue in parallel;
output stores on the scalar queue.
"""

import numpy as np
import ml_dtypes
from contextlib import ExitStack

import concourse.bass as bass
import concourse.bacc as bacc
import concourse.mybir as mybir
import concourse.tile as tile
from concourse.bass_utils import run_bass_kernel_spmd

B, H, W, C = 16, 256, 256, 36
NCORES = 8
BPC = B // NCORES  # batches per core
W2 = 2 * W
H2 = 2 * H
F32 = mybir.dt.float32
BF16 = mybir.dt.bfloat16

SMOOTH = [0.0, 0.0, 1.0 / 16.0, 0.5, 14.0 / 16.0, 0.5, 1.0 / 16.0, 0.0, 0.0]
EVEN = [-1.0 / 128.0, -1.0 / 16.0, -10.0 / 64.0, -7.0 / 16.0, 85.0 / 64.0,
        -7.0 / 16.0, -10.0 / 64.0, -1.0 / 16.0, -1.0 / 128.0]
ODD = [1.0 / 256.0, 1.0 / 32.0, 15.0 / 128.0, 17.0 / 32.0, 0.0,
       -17.0 / 32.0, -15.0 / 128.0, -1.0 / 32.0, -1.0 / 256.0]

# Which 128-row k-tiles of u2 feed each 128-col h2 output block
# (out block n depends on input rows [64n-2, 64n+65]).
KTS = {0: (0,), 1: (0, 1), 2: (0, 1), 3: (1,)}
# Stage-X 70-row input windows per 128-col w2 output block.
W0 = [0, 62, 124, 186]
KW = 70
HKW = KW // 2  # 35

# x row layout: free = (by 3, g2 4, h 128) per h-half = 1536 elems; pad to
# 1600 and load only 1568 so descriptors (3136 B) never coalesce.
FREE = 1536
FREEP = 1600
FREEL = 1568


def _build_operator_full():
    """[3 bands, 256 in-rows, 512 out-cols] float64 folded operator."""
    inv = np.array([SMOOTH, EVEN, ODD], dtype=np.float64)
    S = 256
    Sp = S + 6
    j = np.arange(Sp)[:, None]
    m = np.arange(2 * S)[None, :]
    t = m + 10 - 2 * j
    valid = (t >= 0) & (t <= 8)
    P = np.zeros((3, Sp, 2 * S))
    for b in range(3):
        P[b][valid] = inv[b][t[valid]]
    # border mask: odd band negated on the 3-wide padded border
    P[2, [0, 1, 2, Sp - 3, Sp - 2, Sp - 1], :] *= -1.0
    # fold symmetric padding: pad[0..2]=x[2],x[1],x[0]; pad[-3:]=x[-1],x[-2],x[-3]
    A = P[:, 3:3 + S].copy()
    A[:, 2] += P[:, 0]
    A[:, 1] += P[:, 1]
    A[:, 0] += P[:, 2]
    A[:, S - 1] += P[:, Sp - 3]
    A[:, S - 2] += P[:, Sp - 2]
    A[:, S - 3] += P[:, Sp - 1]
    return A


def _build_ay():
    """Stage-Y operator: [3 bands, 2 ktiles, 128 in-rows, 512 out-cols] bf16."""
    A = _build_operator_full()
    return np.ascontiguousarray(
        A.reshape(3, 2, 128, 512).astype(ml_dtypes.bfloat16))


def _build_ax():
    """Stage-X chunked operator: [4 w2blk, 2 chunks, 105, 128] bf16.

    chunk0 rows = (bx=0, win 70) + (bx=1, win[:35]);
    chunk1 rows = (bx=1, win[35:]) + (bx=2, win 70).
    """
    A = _build_operator_full()
    out = np.zeros((4, 2, KW + HKW, 128), np.float64)
    for n in range(4):
        w = slice(W0[n], W0[n] + KW)
        cols = slice(n * 128, (n + 1) * 128)
        out[n, 0, :KW] = A[0, w, cols]
        out[n, 0, KW:] = A[1, W0[n]:W0[n] + HKW, cols]
        out[n, 1, :HKW] = A[1, W0[n] + HKW:W0[n] + KW, cols]
        out[n, 1, HKW:] = A[2, w, cols]
    return np.ascontiguousarray(out.astype(ml_dtypes.bfloat16))


def _build_program(repeat=1):
    nc = bacc.Bacc("TRN2", target_bir_lowering=False)
    # x: [b, hhalf, w2blk, chunk, 105 rows, FREEP] bf16, row = (by,g2,h128)
    x = nc.declare_dram_parameter("x", [BPC, 2, 4, 2, KW + HKW, FREEP], BF16,
                                  isOutput=False)
    a_y = nc.declare_dram_parameter("a_y", [3, 2, 128, 512], BF16,
                                    isOutput=False)
    a_x = nc.declare_dram_parameter("a_x", [4, 2, KW + HKW, 128], BF16,
                                    isOutput=False)
    ident = nc.declare_dram_parameter("ident", [128, 128], BF16,
                                      isOutput=False)
    # out rows are (g2, w2) -- host swaps to (w2, g2) and upcasts
    out = nc.declare_dram_parameter("out", [BPC, H2, 4, W2], BF16,
                                    isOutput=True)

    with tile.TileContext(nc) as tc, ExitStack() as ctx:
        const = ctx.enter_context(tc.tile_pool(name="const", bufs=1))
        xpool = ctx.enter_context(tc.tile_pool(name="xp", bufs=6))
        tpool = ctx.enter_context(tc.tile_pool(name="tp", bufs=10))
        upool = ctx.enter_context(tc.tile_pool(name="up", bufs=8))
        opool = ctx.enter_context(tc.tile_pool(name="op", bufs=3))
        psX = ctx.enter_context(tc.tile_pool(name="psX", bufs=2, space="PSUM"))
        psT = ctx.enter_context(tc.tile_pool(name="psT", bufs=2, space="PSUM"))
        psY = ctx.enter_context(tc.tile_pool(name="psY", bufs=2, space="PSUM"))

        # ---- constants: scalar-engine DMA queue (parallel to x stream)
        ax_sb = {}
        for n in range(4):
            for c in range(2):
                t = const.tile([KW + HKW, 128], BF16, name=f"ax_{n}_{c}",
                               tag=f"ax_{n}_{c}")
                nc.scalar.dma_start(t[:], a_x[n, c])
                ax_sb[n, c] = t
        ident_sb = const.tile([128, 128], BF16, name="ident_sb", tag="ident")
        nc.scalar.dma_start(ident_sb[:], ident[:])
        ay_sb = {}
        for by in range(3):
            for kt in range(2):
                t = const.tile([128, 512], BF16, name=f"ay_{by}_{kt}",
                               tag=f"ay_{by}_{kt}")
                nc.scalar.dma_start(t[:], a_y[by, kt])
                ay_sb[by, kt] = t

        # ---- input loads: sync-engine queue, exact consumption order
        xts = {}
        for rep in range(repeat):
            for b in range(BPC):
                for hh in range(2):
                    for n in range(4):
                        xt = xpool.tile([KW + HKW, 2 * FREEL], BF16,
                                        name=f"x_{rep}_{b}_{hh}_{n}",
                                        tag="x")
                        dst = xt.rearrange("p (c f) -> p c f", c=2)
                        srcv = x[b, hh, n].rearrange(
                            "c p f -> p c f")[:, :, 0:FREEL]
                        nc.sync.dma_start(dst, srcv)
                        xts[rep, b, hh, n] = xt

        def vcopy(dst, src):
            nc.vector.tensor_copy(out=dst, in_=src)

        def scopy(dst, src):
            nc.scalar.copy(out=dst, in_=src)

        for rep in range(repeat):
          for b in range(BPC):
            rb = rep * BPC + b
            tsb = {}   # (hh, w2t) -> [128 w2, (q 12, h 128)]
            u2 = {}    # (by, kt)  -> [128 h, (g2 4, w2 512)]

            def stage_x(hh):
                for n in range(4):
                    xv = xts[rep, b, hh, n].rearrange(
                        "p (c f) -> p c f", c=2)
                    tt = tpool.tile([128, FREE], BF16,
                                    name=f"t_{rb}_{hh}_{n}",
                                    tag="t")
                    for t3 in range(3):
                        ps = psX.tile([128, 512], F32,
                                      name=f"psX_{rb}_{hh}_{n}_{t3}",
                                      tag="psX")
                        for c in range(2):
                            rhs = xv[:, c, t3 * 512:(t3 + 1) * 512]
                            nc.tensor.matmul(ps[:], ax_sb[n, c][:], rhs,
                                             start=(c == 0), stop=(c == 1))
                        scopy(tt[:, t3 * 512:(t3 + 1) * 512], ps[:])
                    tsb[hh, n] = tt

            def stage_t(kt):
                for by in range(3):
                    ut = upool.tile([128, 2048], BF16,
                                    name=f"u_{rb}_{by}_{kt}",
                                    tag="u")
                    for g2 in range(4):
                        q = by * 4 + g2
                        pt = psT.tile([128, 512], BF16,
                                      name=f"psT_{rb}_{kt}_{by}_{g2}",
                                      tag="psT")
                        for w2t in range(4):
                            in_ = tsb[kt, w2t][:, q * 128:(q + 1) * 128]
                            nc.tensor.transpose(
                                pt[:, w2t * 128:(w2t + 1) * 128],
                                in_, ident_sb[:])
                        # u2 free layout (g2, w2): contiguous dst -> 2x DVE
                        vcopy(ut[:, g2 * 512:(g2 + 1) * 512], pt[:])
                    u2[by, kt] = ut

            def stage_y(n):
                osb = opool.tile([128, 4 * 512], BF16,
                                 name=f"osb_{rb}_{n}", tag="osb")
                for g2 in range(4):
                    ps = psY.tile([128, 512], F32,
                                  name=f"psY_{rb}_{n}_{g2}", tag="psY")
                    mms = [(by, kt) for by in range(3) for kt in KTS[n]]
                    for i, (by, kt) in enumerate(mms):
                        lhsT = ay_sb[by, kt][:, n * 128:(n + 1) * 128]
                        rhs = u2[by, kt][:, g2 * 512:(g2 + 1) * 512]
                        nc.tensor.matmul(ps[:], lhsT, rhs,
                                         start=(i == 0),
                                         stop=(i == len(mms) - 1))
                    vcopy(osb[:, g2 * 512:(g2 + 1) * 512], ps[:])
                dstd = out[b, n * 128:(n + 1) * 128, :, :].rearrange(
                    "h g w -> h (g w)")
                nc.scalar.dma_start(dstd, osb[:])

            stage_x(0)
            stage_t(0)
            stage_y(0)
            stage_x(1)
            stage_t(1)
            stage_y(1)
            stage_y(2)
            stage_y(3)
    nc.compile()
    return nc


_PROGRAMS = {}


def _get_program(repeat=1):
    if repeat not in _PROGRAMS:
        _PROGRAMS[repeat] = _build_program(repeat)
    return _PROGRAMS[repeat]


def _host_inputs(inputs):
    ax = _build_ax()
    ay = _build_ay()
    identity = np.ascontiguousarray(np.eye(128, dtype=ml_dtypes.bfloat16))
    # [B,H,W,C] c = 9*g2 + 3*by + bx -> xt [b, bx, w, by, g2, h] bf16
    xt = inputs.reshape(B, H, W, 4, 3, 3).transpose(0, 5, 2, 4, 3, 1)
    xt = np.ascontiguousarray(xt).astype(ml_dtypes.bfloat16)
    # band-stacked chunk windows, h-halved:
    # xw [b, hh, w2blk, chunk, 105, (by, g2, h128)] (+pad)
    xw = np.zeros((B, 2, 4, 2, KW + HKW, FREEP), dtype=ml_dtypes.bfloat16)
    xr = xw[..., :FREE].reshape(B, 2, 4, 2, KW + HKW, 3, 4, 128)
    for n in range(4):
        w = slice(W0[n], W0[n] + KW)
        wa = slice(W0[n], W0[n] + HKW)
        wb = slice(W0[n] + HKW, W0[n] + KW)
        for hh in range(2):
            h = slice(hh * 128, (hh + 1) * 128)
            # xt slice -> [b, w, by, g2, h]
            xr[:, hh, n, 0, :KW] = xt[:, 0, w, :, :, h]
            xr[:, hh, n, 0, KW:] = xt[:, 1, wa, :, :, h]
            xr[:, hh, n, 1, :HKW] = xt[:, 1, wb, :, :, h]
            xr[:, hh, n, 1, HKW:] = xt[:, 2, w, :, :, h]
    shards = xw.reshape(NCORES, BPC, 2, 4, 2, KW + HKW, FREEP)
    return [{"x": np.ascontiguousarray(shards[c]), "a_y": ay, "a_x": ax,
             "ident": identity} for c in range(NCORES)]


def _run(inputs, trace=False, tmpdir=None, repeat=1):
    """Returns (full output [16,512,512,4], BassKernelResults)."""
    inputs = np.ascontiguousarray(np.asarray(inputs, dtype=np.float32))
    assert inputs.shape == (B, H, W, C), inputs.shape
    nc = _get_program(repeat)
    in_maps = _host_inputs(inputs)
    res = run_bass_kernel_spmd(nc, in_maps, core_ids=list(range(NCORES)),
                               trace=trace, tmpdir=tmpdir)
    outs = [np.asarray(res.results[c]["out"]) for c in range(NCORES)]
    full = np.concatenate(outs, axis=0)  # [16, 512, 4, 512] bf16
    full = full.transpose(0, 1, 3, 2).astype(np.float32)
    return np.ascontiguousarray(full), res


def kernel(inputs):
    full, _ = _run(inputs)
    return full


# revision 9
# speedup vs baseline: 1.6192x; 1.0226x over previous
"""Inverse separable wavelet synthesis (stride-2 transposed conv, 9 taps,
36 -> 12 -> 4 channels, 256x256 -> 512x512) on 8 trn2 NeuronCores.

X-FIRST dense-operator formulation (v3).  Both passes are matmuls against
the same host-precomputed banded operator A [256 in, 512 out] (one per
wavelet band, symmetric padding + border-mask sign folded in).  All
coefficients are dyadic rationals exact in bf16.

Pass 1 (X, along width) runs FIRST (reference order), contraction over
(bx band, w window) jointly: the host stacks the 3 bands' 70-row windows
into two 105-row chunks, so each 128-col w2 output block needs only TWO
matmul pumps (vs 4.5 for tile-aligned k + per-band pumps).

  t[w2, (by,g2p,h,g2s)] = sum_{bx,w} A[bx,w,w2] x[h,w,c]   (48 mm/img)

Mid transpose on the PE (identity trick): output channels are packed in
g2-PAIRS as one fp32 element (2 bf16s), so each 128x128 fp32 transpose
moves two channels at once -> 48 transposes/img instead of 96.

  u2[by,kt][h, (g2p,w2,g2s)] = t^T                         (48 tr/img)

Pass 2 (Y, along height) contracts h k-tiles (KTS banding, 1-2 tiles per
128-row h2 block) and lands DIRECTLY in output row layout -> no output
transpose at all:

  out[h2, (g2p,w2,g2s)] = sum_{by,kt} A[by,kt,h2] u2       (72 mm/img)

Output is stored bf16 [b, h2, g2p, w2, g2s] (halves store bytes); the
host reorders channels and upcasts to fp32.

DMA: input is host-materialized as band-stacked chunk windows, split by
h-half so compute starts after ~1.3 MB instead of ~5 MB; all x loads go
in exact consumption order on the sync-engine queue (the baseline lost
~40 us to a later-needed load winning DMA-engine arbitration over the
first-needed one).  3 KB descriptors spread over all 16 DMA engines and
reach ~300 GB/s (vs ~200 for 18 KB ones).  Constants load on the
scalar-engine queue in parallel; output stores ride the sync queue
behind the inputs, issued per psY quarter-tile to shorten the tail.
"""

import numpy as np
import ml_dtypes
from contextlib import ExitStack

import concourse.bass as bass
import concourse.bacc as bacc
import concourse.mybir as mybir
import concourse.tile as tile
from concourse.bass_utils import run_bass_kernel_spmd

B, H, W, C = 16, 256, 256, 36
NCORES = 8
BPC = B // NCORES  # batches per core
W2 = 2 * W
H2 = 2 * H
F32 = mybir.dt.float32
BF16 = mybir.dt.bfloat16

SMOOTH = [0.0, 0.0, 1.0 / 16.0, 0.5, 14.0 / 16.0, 0.5, 1.0 / 16.0, 0.0, 0.0]
EVEN = [-1.0 / 128.0, -1.0 / 16.0, -10.0 / 64.0, -7.0 / 16.0, 85.0 / 64.0,
        -7.0 / 16.0, -10.0 / 64.0, -1.0 / 16.0, -1.0 / 128.0]
ODD = [1.0 / 256.0, 1.0 / 32.0, 15.0 / 128.0, 17.0 / 32.0, 0.0,
       -17.0 / 32.0, -15.0 / 128.0, -1.0 / 32.0, -1.0 / 256.0]

# Which 128-row k-tiles of u2 feed each 128-col h2 output block
# (out block n depends on input rows [64n-2, 64n+65]).
KTS = {0: (0,), 1: (0, 1), 2: (0, 1), 3: (1,)}
# Stage-X 70-row input windows per 128-col w2 output block.
W0 = [0, 62, 124, 186]
KW = 70
HKW = KW // 2  # 35

# x row layout: free = (by 3, g2p 2, h 128, g2s 2) per h-half = 1536 elems;
# pad to 1600 and load only 1568 so descriptors (3136 B) never coalesce.
FREE = 1536
FREEP = 1600
FREEL = 1568


def _build_operator_full():
    """[3 bands, 256 in-rows, 512 out-cols] float64 folded operator."""
    inv = np.array([SMOOTH, EVEN, ODD], dtype=np.float64)
    S = 256
    Sp = S + 6
    j = np.arange(Sp)[:, None]
    m = np.arange(2 * S)[None, :]
    t = m + 10 - 2 * j
    valid = (t >= 0) & (t <= 8)
    P = np.zeros((3, Sp, 2 * S))
    for b in range(3):
        P[b][valid] = inv[b][t[valid]]
    # border mask: odd band negated on the 3-wide padded border
    P[2, [0, 1, 2, Sp - 3, Sp - 2, Sp - 1], :] *= -1.0
    # fold symmetric padding: pad[0..2]=x[2],x[1],x[0]; pad[-3:]=x[-1],x[-2],x[-3]
    A = P[:, 3:3 + S].copy()
    A[:, 2] += P[:, 0]
    A[:, 1] += P[:, 1]
    A[:, 0] += P[:, 2]
    A[:, S - 1] += P[:, Sp - 3]
    A[:, S - 2] += P[:, Sp - 2]
    A[:, S - 3] += P[:, Sp - 1]
    return A


def _build_ay():
    """Stage-Y operator: [3 bands, 2 ktiles, 128 in-rows, 512 out-cols] bf16."""
    A = _build_operator_full()
    return np.ascontiguousarray(
        A.reshape(3, 2, 128, 512).astype(ml_dtypes.bfloat16))


def _build_ax():
    """Stage-X chunked operator: [4 w2blk, 2 chunks, 105, 128] bf16.

    chunk0 rows = (bx=0, win 70) + (bx=1, win[:35]);
    chunk1 rows = (bx=1, win[35:]) + (bx=2, win 70).
    """
    A = _build_operator_full()
    out = np.zeros((4, 2, KW + HKW, 128), np.float64)
    for n in range(4):
        w = slice(W0[n], W0[n] + KW)
        cols = slice(n * 128, (n + 1) * 128)
        out[n, 0, :KW] = A[0, w, cols]
        out[n, 0, KW:] = A[1, W0[n]:W0[n] + HKW, cols]
        out[n, 1, :HKW] = A[1, W0[n] + HKW:W0[n] + KW, cols]
        out[n, 1, HKW:] = A[2, w, cols]
    return np.ascontiguousarray(out.astype(ml_dtypes.bfloat16))


def _build_program(repeat=1):
    nc = bacc.Bacc("TRN2", target_bir_lowering=False)
    # x: [b, hhalf, w2blk, chunk, 105 rows, FREEP] bf16
    x = nc.declare_dram_parameter("x", [BPC, 2, 4, 2, KW + HKW, FREEP], BF16,
                                  isOutput=False)
    a_y = nc.declare_dram_parameter("a_y", [3, 2, 128, 512], BF16,
                                    isOutput=False)
    a_x = nc.declare_dram_parameter("a_x", [4, 2, KW + HKW, 128], BF16,
                                    isOutput=False)
    identf = nc.declare_dram_parameter("identf", [128, 128], F32,
                                       isOutput=False)
    # out rows are (g2p, w2, g2s) -- host reorders and upcasts
    out = nc.declare_dram_parameter("out", [BPC, H2, 2, W2, 2], BF16,
                                    isOutput=True)

    with tile.TileContext(nc) as tc, ExitStack() as ctx:
        const = ctx.enter_context(tc.tile_pool(name="const", bufs=1))
        xpool = ctx.enter_context(tc.tile_pool(name="xp", bufs=6))
        tpool = ctx.enter_context(tc.tile_pool(name="tp", bufs=10))
        upool = ctx.enter_context(tc.tile_pool(name="up", bufs=8))
        opool = ctx.enter_context(tc.tile_pool(name="op", bufs=3))
        psX = ctx.enter_context(tc.tile_pool(name="psX", bufs=2, space="PSUM"))
        psT = ctx.enter_context(tc.tile_pool(name="psT", bufs=2, space="PSUM"))
        psY = ctx.enter_context(tc.tile_pool(name="psY", bufs=2, space="PSUM"))

        # ---- constants: scalar-engine DMA queue (parallel to x stream)
        ax_sb = {}
        for n in range(4):
            for c in range(2):
                t = const.tile([KW + HKW, 128], BF16, name=f"ax_{n}_{c}",
                               tag=f"ax_{n}_{c}")
                nc.scalar.dma_start(t[:], a_x[n, c])
                ax_sb[n, c] = t
        identf_sb = const.tile([128, 128], F32, name="identf_sb", tag="idf")
        nc.scalar.dma_start(identf_sb[:], identf[:])
        ay_sb = {}
        for by in range(3):
            for kt in range(2):
                t = const.tile([128, 512], BF16, name=f"ay_{by}_{kt}",
                               tag=f"ay_{by}_{kt}")
                nc.scalar.dma_start(t[:], a_y[by, kt])
                ay_sb[by, kt] = t

        # ---- input loads: sync-engine queue, exact consumption order
        xts = {}
        for rep in range(repeat):
            for b in range(BPC):
                for hh in range(2):
                    for n in range(4):
                        xt = xpool.tile([KW + HKW, 2 * FREEL], BF16,
                                        name=f"x_{rep}_{b}_{hh}_{n}",
                                        tag="x")
                        dst = xt.rearrange("p (c f) -> p c f", c=2)
                        srcv = x[b, hh, n].rearrange(
                            "c p f -> p c f")[:, :, 0:FREEL]
                        nc.sync.dma_start(dst, srcv)
                        xts[rep, b, hh, n] = xt

        def vcopy(dst, src):
            nc.vector.tensor_copy(out=dst, in_=src)

        def scopy(dst, src):
            nc.scalar.copy(out=dst, in_=src)

        for rep in range(repeat):
          for b in range(BPC):
            rb = rep * BPC + b
            tsb = {}   # (hh, w2t) -> [128 w2, (by 3, g2p 2, h 128, g2s 2)]
            u2 = {}    # (by, kt)  -> [128 h, (g2p 2, w2 512, g2s 2)]

            def stage_x(hh):
                for n in range(4):
                    xv = xts[rep, b, hh, n].rearrange(
                        "p (c f) -> p c f", c=2)
                    tt = tpool.tile([128, FREE], BF16,
                                    name=f"t_{rb}_{hh}_{n}", tag="t")
                    for t3 in range(3):
                        ps = psX.tile([128, 512], F32,
                                      name=f"psX_{rb}_{hh}_{n}_{t3}",
                                      tag="psX")
                        for c in range(2):
                            rhs = xv[:, c, t3 * 512:(t3 + 1) * 512]
                            nc.tensor.matmul(ps[:], ax_sb[n, c][:], rhs,
                                             start=(c == 0), stop=(c == 1))
                        scopy(tt[:, t3 * 512:(t3 + 1) * 512], ps[:])
                    tsb[hh, n] = tt

            def stage_t(kt):
                # fp32-packed transposes: one 128x128 fp32 transpose moves a
                # g2-PAIR of bf16 channels at once.
                for by in range(3):
                    ut = upool.tile([128, 2048], BF16,
                                    name=f"u_{rb}_{by}_{kt}", tag="u")
                    for gp in range(2):
                        qp = by * 2 + gp
                        pt = psT.tile([128, 512], F32,
                                      name=f"psT_{rb}_{kt}_{by}_{gp}",
                                      tag="psT")
                        for w2t in range(4):
                            in_ = tsb[kt, w2t][:].bitcast(F32)[
                                :, qp * 128:(qp + 1) * 128]
                            nc.tensor.transpose(
                                pt[:, w2t * 128:(w2t + 1) * 128],
                                in_, identf_sb[:])
                        vcopy(ut[:, gp * 1024:(gp + 1) * 1024],
                              pt[:].bitcast(BF16))
                    u2[by, kt] = ut

            def stage_y(n):
                osb = opool.tile([128, 4 * 512], BF16,
                                 name=f"osb_{rb}_{n}", tag="osb")
                ov = out[b, n * 128:(n + 1) * 128].rearrange(
                    "h gp w gs -> h (gp w gs)")
                for q in range(4):
                    ps = psY.tile([128, 512], F32,
                                  name=f"psY_{rb}_{n}_{q}", tag="psY")
                    mms = [(by, kt) for by in range(3) for kt in KTS[n]]
                    for i, (by, kt) in enumerate(mms):
                        lhsT = ay_sb[by, kt][:, n * 128:(n + 1) * 128]
                        rhs = u2[by, kt][:, q * 512:(q + 1) * 512]
                        nc.tensor.matmul(ps[:], lhsT, rhs,
                                         start=(i == 0),
                                         stop=(i == len(mms) - 1))
                    vcopy(osb[:, q * 512:(q + 1) * 512], ps[:])
                    # store each quarter as soon as its copy lands
                    nc.sync.dma_start(ov[:, q * 512:(q + 1) * 512],
                                      osb[:, q * 512:(q + 1) * 512])

            stage_x(0)
            stage_t(0)
            stage_y(0)
            stage_x(1)
            stage_t(1)
            stage_y(1)
            stage_y(2)
            stage_y(3)
    nc.compile()
    return nc


_PROGRAMS = {}


def _get_program(repeat=1):
    if repeat not in _PROGRAMS:
        _PROGRAMS[repeat] = _build_program(repeat)
    return _PROGRAMS[repeat]


def _host_inputs(inputs):
    ax = _build_ax()
    ay = _build_ay()
    identity = np.ascontiguousarray(np.eye(128, dtype=np.float32))
    # c = 18*g2p + 9*g2s + 3*by + bx  ->  xt [b, bx, w, by, g2p, h, g2s] bf16
    xt = inputs.reshape(B, H, W, 2, 2, 3, 3).transpose(0, 6, 2, 5, 3, 1, 4)
    xt = np.ascontiguousarray(xt).astype(ml_dtypes.bfloat16)
    # band-stacked chunk windows, h-halved:
    # xw [b, hh, w2blk, chunk, 105, (by, g2p, h128, g2s)] (+pad)
    xw = np.zeros((B, 2, 4, 2, KW + HKW, FREEP), dtype=ml_dtypes.bfloat16)
    xr = xw[..., :FREE].reshape(B, 2, 4, 2, KW + HKW, 3, 2, 128, 2)
    for n in range(4):
        w = slice(W0[n], W0[n] + KW)
        wa = slice(W0[n], W0[n] + HKW)
        wb = slice(W0[n] + HKW, W0[n] + KW)
        for hh in range(2):
            h = slice(hh * 128, (hh + 1) * 128)
            # xt slice -> [b, w, by, g2p, h, g2s]
            xr[:, hh, n, 0, :KW] = xt[:, 0, w, :, :, h]
            xr[:, hh, n, 0, KW:] = xt[:, 1, wa, :, :, h]
            xr[:, hh, n, 1, :HKW] = xt[:, 1, wb, :, :, h]
            xr[:, hh, n, 1, HKW:] = xt[:, 2, w, :, :, h]
    shards = xw.reshape(NCORES, BPC, 2, 4, 2, KW + HKW, FREEP)
    return [{"x": np.ascontiguousarray(shards[c]), "a_y": ay, "a_x": ax,
             "identf": identity} for c in range(NCORES)]


def _run(inputs, trace=False, tmpdir=None, repeat=1):
    """Returns (full output [16,512,512,4], BassKernelResults)."""
    inputs = np.ascontiguousarray(np.asarray(inputs, dtype=np.float32))
    assert inputs.shape == (B, H, W, C), inputs.shape
    nc = _get_program(repeat)
    in_maps = _host_inputs(inputs)
    res = run_bass_kernel_spmd(nc, in_maps, core_ids=list(range(NCORES)),
                               trace=trace, tmpdir=tmpdir)
    outs = [np.asarray(res.results[c]["out"]) for c in range(NCORES)]
    full = np.concatenate(outs, axis=0)  # [16, 512, 2, 512, 2] bf16
    # (g2p, w2, g2s) -> (w2, g2p, g2s) = (w2, g2)
    full = full.transpose(0, 1, 3, 2, 4).reshape(B, H2, W2, 4)
    return np.ascontiguousarray(full.astype(np.float32)), res


def kernel(inputs):
    full, _ = _run(inputs)
    return full
